# revision 10
# baseline (speedup 1.0000x reference)
"""Trainium2 Bass kernel for nn_AtenMatmulQMixedSigni8.

Reference computation:
    xf = (x_int8  - (-66)) * x_scale      # [7, 8, 512, 1024]
    yf = (y_uint8 - 160)   * y_scale      # [8, 1024, 512]
    out = einsum('gbmk,bkn->gbmn', xf, yf)  # [7, 8, 512, 512] f32

Strategy:
  - Shard data-parallel over the B=8 batch axis: core b gets x[:, b], y[b],
    produces out[:, b]. No collectives.
  - The centered integer values (x+66) in [-62, 193] and (y-160) in
    [-160, 95] are exactly representable in bf16, so the matmul runs at
    full bf16 TensorEngine rate and is numerically exact (fp32 PSUM
    accumulation); the only epilogue is a multiply by x_scale*y_scale.
  - x is pre-transposed on the host to [K, M] per (g, b) so its tiles load
    directly as the stationary (lhsT) operand with no on-chip transpose.
  - Raw Bass (explicit engine programs + semaphores): the Tile layer's
    generated sync exceeds walrus' per-instruction sync-wait limits for
    this DMA pattern (DMA descriptors allow one wait; the kernel-tail
    drain has a small cap). With raw Bass every wait is its own sequencer
    instruction, so no limits apply.

Pipeline per core:
  sync engine   : 4 input DMAs (y, then x in 3 groups), one semaphore each
  tensor engine : 28 matmul groups (g,m), 8 accumulating matmuls each,
                  rotating through the 8 PSUM banks
  scalar engine : epilogue per group: PSUM * scale -> SBUF f32
  gpsimd engine : one store DMA per g (after its 4 epilogues), final wait
"""

import os
import sys

sys.path.insert(0, "/opt/trn_rl_repo")

import numpy as np
import ml_dtypes

G, B, M, K, N = 7, 8, 512, 1024, 512
P = 128
X_ZP = -66
Y_ZP = 160

KO = K // P   # 8 k-tiles per matmul group
MO = M // P   # 4 m-tiles (groups) per g
NBANK = 8     # PSUM banks
XGROUPS = [(0, 3), (3, 5), (5, 7)]


def _build_graph(scale: float):
    import concourse.bass as bass
    import concourse.mybir as mybir

    nc = bass.Bass()

    xt = nc.declare_dram_parameter("xt", [G, K, M], mybir.dt.bfloat16, isOutput=False)
    y = nc.declare_dram_parameter("y", [K, N], mybir.dt.bfloat16, isOutput=False)
    out = nc.declare_dram_parameter("out", [G, M, N], mybir.dt.float32, isOutput=True)

    with (
        nc.sbuf_tensor("ysb", [P, KO, N], mybir.dt.bfloat16) as ysb,
        nc.sbuf_tensor("xsb", [P, G * KO, M], mybir.dt.bfloat16) as xsb,
        nc.sbuf_tensor("osb", [P, G * MO, N], mybir.dt.float32) as osb,
        nc.psum_tensor("ps", [P, NBANK, N], mybir.dt.float32) as ps,
        nc.semaphore("ysem") as ysem,
        nc.semaphore("xsem0") as xsem0,
        nc.semaphore("xsem1") as xsem1,
        nc.semaphore("xsem2") as xsem2,
        nc.semaphore("pesem") as pesem,
        nc.semaphore("actsem") as actsem,
        nc.semaphore("outsem") as outsem,
        nc.Block() as block,
    ):
        xsems = [xsem0, xsem1, xsem2]

        @block.sync
        def _(sync):
            sync.dma_start(
                ysb[:], y.rearrange("(ko p) n -> p ko n", p=P)
            ).then_inc(ysem, 16)
            for j, (s, e) in enumerate(XGROUPS):
                sync.dma_start(
                    xsb[:, s * KO : e * KO, :],
                    xt[s:e].rearrange("g (ko p) m -> p (g ko) m", p=P),
                ).then_inc(xsems[j], 16)

        @block.tensor
        def _(tensor):
            tensor.wait_ge(ysem, 16)
            i = 0
            for j, (s, e) in enumerate(XGROUPS):
                tensor.wait_ge(xsems[j], 16)
                for g in range(s, e):
                    for m in range(MO):
                        if i >= NBANK:
                            # PSUM bank reuse: epilogue of group i-8 done.
                            tensor.wait_ge(actsem, i - NBANK + 1)
                        mm = None
                        for k in range(KO):
                            mm = tensor.matmul(
                                ps[:, i % NBANK, :],
                                xsb[:, g * KO + k, m * P : (m + 1) * P],
                                ysb[:, k, :],
                                start=(k == 0),
                                stop=(k == KO - 1),
                            )
                        mm.then_inc(pesem, 1)
                        i += 1

        @block.scalar
        def _(scalar):
            i = 0
            for g in range(G):
                for m in range(MO):
                    scalar.wait_ge(pesem, i + 1)
                    scalar.mul(
                        osb[:, g * MO + m, :], ps[:, i % NBANK, :], scale
                    ).then_inc(actsem, 1)
                    i += 1

        @block.gpsimd
        def _(gpsimd):
            for g in range(G):
                gpsimd.wait_ge(actsem, MO * (g + 1))
                gpsimd.dma_start(
                    out[g].rearrange("(mo p) n -> p mo n", p=P),
                    osb[:, g * MO : (g + 1) * MO, :],
                ).then_inc(outsem, 16)
            gpsimd.wait_ge(outsem, 16 * G)

    return nc


def kernel(x, y, x_scale, y_scale):
    from concourse.bass_utils import run_bass_kernel_spmd

    x = np.asarray(x)
    y = np.asarray(y)
    scale = float(np.float32(x_scale) * np.float32(y_scale))

    # Center to remove zero points; values stay small integers -> exact bf16.
    # x: [G, B, M, K] -> transpose to [B, G, K, M] (k-major for lhsT tiles).
    xc = (x.astype(np.int16) - np.int16(X_ZP)).astype(np.float32)
    xt = np.ascontiguousarray(xc.transpose(1, 0, 3, 2)).astype(ml_dtypes.bfloat16)
    yc = (y.astype(np.int16) - np.int16(Y_ZP)).astype(np.float32).astype(
        ml_dtypes.bfloat16
    )

    nc = _build_graph(scale)

    in_maps = [{"xt": xt[b], "y": yc[b]} for b in range(B)]
    core_ids = list(range(B))

    kwargs = {}
    if os.environ.get("BASS_KERNEL_TRACE"):
        # Profiling path (test.py only): install the NTFF hook that the
        # image's antenv lacks, and skip the fishshare artifact upload.
        import types
        import antenv
        from concourse import bass_utils as _bu
        from trn_agent_boot import trn_boot as _tb

        mod = types.ModuleType("antenv.axon_hooks")
        _hook_box = {}
        mod.set_axon_ntff_profile_hook = lambda h: _hook_box.update(h=h)
        mod.get_axon_ntff_profile_hook = lambda: _hook_box.get("h")
        sys.modules["antenv.axon_hooks"] = mod
        antenv.axon_hooks = mod
        mod.set_axon_ntff_profile_hook(
            _tb._ntff_profile_via_ctypes("/opt/axon/libaxon_pjrt.so")
        )
        _bu.upload_artifacts = lambda tmpdir: f"file://{tmpdir}"
        tdir = os.environ.get("BASS_KERNEL_TRACE_DIR") or None
        kwargs = dict(trace=True, tmpdir=tdir)

    res = run_bass_kernel_spmd(nc, in_maps, core_ids, **kwargs)
    if os.environ.get("BASS_KERNEL_TRACE"):
        print(f"HW exec time: {res.exec_time_ns} ns")

    out = np.empty((G, B, M, N), dtype=np.float32)
    for b in range(B):
        out[:, b] = res.results[b]["out"]
    return out


if __name__ == "__main__":
    rng = np.random.default_rng(0)
    x = rng.integers(-128, 128, size=(G, B, M, K), dtype=np.int32).astype(np.int8)
    y = rng.integers(0, 256, size=(B, K, N), dtype=np.int32).astype(np.uint8)
    out = kernel(x, y, np.float32(0.03), np.float32(0.025))
    ref = np.einsum(
        "gbmk,bkn->gbmn",
        (x.astype(np.float32) + 66.0) * 0.03,
        (y.astype(np.float32) - 160.0) * 0.025,
    )
    err = np.abs(out - ref).max() / max(np.abs(ref).max(), 1e-9)
    print("max rel err:", err)


# revision 12
# speedup vs baseline: 1.1239x; 1.1239x over previous
"""Trainium2 Bass kernel for nn_AtenMatmulQMixedSigni8.

Reference computation:
    xf = (x_int8  - (-66)) * x_scale      # [7, 8, 512, 1024]
    yf = (y_uint8 - 160)   * y_scale      # [8, 1024, 512]
    out = einsum('gbmk,bkn->gbmn', xf, yf)  # [7, 8, 512, 512] f32

Strategy:
  - Shard data-parallel over the B=8 batch axis: core b gets x[:, b], y[b],
    produces out[:, b]. No collectives.
  - The centered integer values (x+66) in [-62, 193] and (y-160) in
    [-160, 95] are exactly representable in bf16, so the matmul runs at
    full bf16 TensorEngine rate and is numerically exact (fp32 PSUM
    accumulation); the only epilogue is a multiply by x_scale*y_scale.
  - Host pre-packs x (transposed to lhsT layout) and y into the exact
    SBUF tile layout (partition-major), so every DMA moves long
    contiguous per-partition runs (8KB+ descriptors). The device writes
    its output in SBUF layout too; the host un-permutes afterwards.
  - Raw Bass (explicit engine programs + semaphores): the Tile layer's
    generated sync exceeds walrus' per-instruction sync-wait limits for
    this DMA pattern. With raw Bass every wait is its own sequencer
    instruction, so no limits apply.

Pipeline per core:
  sync engine   : input DMAs in issue order y, x[g=0], then x g-pairs —
                  the ring is FIFO so the first-needed tiles land first
  tensor engine : 28 matmul groups (g,m), 8 accumulating matmuls each,
                  rotating through the 8 PSUM banks
  scalar engine : per group: epilogue (PSUM * scale -> SBUF f32), then
                  the store DMA on its own HWDGE ring (program order —
                  no cross-engine hop), then a final completion wait
"""

import os
import sys

sys.path.insert(0, "/opt/trn_rl_repo")

import numpy as np
import ml_dtypes

G, B, M, K, N = 7, 8, 512, 1024, 512
P = 128
X_ZP = -66
Y_ZP = 160

KO = K // P   # 8 k-tiles per matmul group
MO = M // P   # 4 m-tiles (groups) per g
NG = G * MO   # 28 matmul groups
NBANK = 8     # PSUM banks
XLOADS = [(0, 1), (1, 3), (3, 5), (5, 7)]  # g-ranges per x load DMA


def _build_graph(scale: float):
    import concourse.bass as bass
    import concourse.mybir as mybir

    nc = bass.Bass()

    # All DRAM tensors are laid out exactly like their SBUF tiles
    # (partition dim outermost), so each DMA is 128 long contiguous runs.
    xd = nc.declare_dram_parameter(
        "xp", [P, G * KO, M], mybir.dt.bfloat16, isOutput=False
    )
    yd = nc.declare_dram_parameter("yp", [P, KO, N], mybir.dt.bfloat16, isOutput=False)
    od = nc.declare_dram_parameter("op", [P, NG, N], mybir.dt.float32, isOutput=True)

    with (
        nc.sbuf_tensor("ysb", [P, KO, N], mybir.dt.bfloat16) as ysb,
        nc.sbuf_tensor("xsb", [P, G * KO, M], mybir.dt.bfloat16) as xsb,
        nc.sbuf_tensor("osb", [P, NG, N], mybir.dt.float32) as osb,
        nc.psum_tensor("ps", [P, NBANK, N], mybir.dt.float32) as ps,
        nc.semaphore("ysem") as ysem,
        nc.semaphore("xsem0") as xsem0,
        nc.semaphore("xsem1") as xsem1,
        nc.semaphore("xsem2") as xsem2,
        nc.semaphore("xsem3") as xsem3,
        nc.semaphore("pesem") as pesem,
        nc.semaphore("actsem") as actsem,
        nc.semaphore("outsem") as outsem,
        nc.Block(no_gpsimd_drain=True) as block,
    ):
        xsems = [xsem0, xsem1, xsem2, xsem3]

        @block.sync
        def _(sync):
            # FIFO ring: y and x[g=0] land first, so PE starts after ~2MB.
            sync.dma_start(ysb[:], yd[:]).then_inc(ysem, 16)
            for j, (s, e) in enumerate(XLOADS):
                sync.dma_start(
                    xsb[:, s * KO : e * KO, :], xd[:, s * KO : e * KO, :]
                ).then_inc(xsems[j], 16)

        @block.tensor
        def _(tensor):
            tensor.wait_ge(ysem, 16)
            i = 0
            for j, (s, e) in enumerate(XLOADS):
                tensor.wait_ge(xsems[j], 16)
                for g in range(s, e):
                    for m in range(MO):
                        if i >= NBANK:
                            # PSUM bank reuse: epilogue of group i-8 done.
                            tensor.wait_ge(actsem, i - NBANK + 1)
                        mm = None
                        for k in range(KO):
                            mm = tensor.matmul(
                                ps[:, i % NBANK, :],
                                xsb[:, g * KO + k, m * P : (m + 1) * P],
                                ysb[:, k, :],
                                start=(k == 0),
                                stop=(k == KO - 1),
                            )
                        mm.then_inc(pesem, 1)
                        i += 1

        @block.scalar
        def _(scalar):
            for i in range(NG):
                scalar.wait_ge(pesem, i + 1)
                scalar.mul(osb[:, i, :], ps[:, i % NBANK, :], scale).then_inc(
                    actsem, 1
                )
                # The DMA doorbell must not ring before the epilogue's SBUF
                # writes land, so gate it on the epilogue's own semaphore.
                scalar.wait_ge(actsem, i + 1)
                scalar.dma_start(od[:, i, :], osb[:, i, :]).then_inc(outsem, 16)
            scalar.wait_ge(outsem, 16 * NG)

    return nc


def kernel(x, y, x_scale, y_scale):
    from concourse.bass_utils import run_bass_kernel_spmd

    x = np.asarray(x)
    y = np.asarray(y)
    scale = float(np.float32(x_scale) * np.float32(y_scale))

    # Center to remove zero points; values stay small integers -> exact
    # bf16. Pack into SBUF layout:
    #   xp[b][p, g*KO + ko, m] = x[g, b, m, ko*P + p] + 66   (lhsT layout)
    #   yp[b][p, ko, n]        = y[b, ko*P + p, n] - 160
    xc = (x.astype(np.int16) - np.int16(X_ZP)).astype(ml_dtypes.bfloat16)
    # [G, B, M, KO, P] -> [B, P, G, KO, M]
    xp = np.ascontiguousarray(
        xc.reshape(G, B, M, KO, P).transpose(1, 4, 0, 3, 2)
    ).reshape(B, P, G * KO, M)
    yc = (y.astype(np.int16) - np.int16(Y_ZP)).astype(ml_dtypes.bfloat16)
    yp = np.ascontiguousarray(yc.reshape(B, KO, P, N).transpose(0, 2, 1, 3))

    nc = _build_graph(scale)

    in_maps = [{"xp": xp[b], "yp": yp[b]} for b in range(B)]
    core_ids = list(range(B))

    kwargs = {}
    if os.environ.get("BASS_KERNEL_TRACE"):
        # Profiling path (test.py only): install the NTFF hook that the
        # image's antenv lacks, and skip the fishshare artifact upload.
        import types
        import antenv
        from concourse import bass_utils as _bu
        from trn_agent_boot import trn_boot as _tb

        mod = types.ModuleType("antenv.axon_hooks")
        _hook_box = {}
        mod.set_axon_ntff_profile_hook = lambda h: _hook_box.update(h=h)
        mod.get_axon_ntff_profile_hook = lambda: _hook_box.get("h")
        sys.modules["antenv.axon_hooks"] = mod
        antenv.axon_hooks = mod
        mod.set_axon_ntff_profile_hook(
            _tb._ntff_profile_via_ctypes("/opt/axon/libaxon_pjrt.so")
        )
        _bu.upload_artifacts = lambda tmpdir: f"file://{tmpdir}"
        tdir = os.environ.get("BASS_KERNEL_TRACE_DIR") or None
        kwargs = dict(trace=True, tmpdir=tdir)

    res = run_bass_kernel_spmd(nc, in_maps, core_ids, **kwargs)
    if os.environ.get("BASS_KERNEL_TRACE"):
        print(f"HW exec time: {res.exec_time_ns} ns")

    # op[b][p, g*MO + mo, n] = out[g, b, mo*P + p, n]
    out = np.empty((G, B, M, N), dtype=np.float32)
    for b in range(B):
        ob = res.results[b]["op"].reshape(P, G, MO, N)
        out[:, b] = ob.transpose(1, 2, 0, 3).reshape(G, M, N)
    return out


if __name__ == "__main__":
    rng = np.random.default_rng(0)
    x = rng.integers(-128, 128, size=(G, B, M, K), dtype=np.int32).astype(np.int8)
    y = rng.integers(0, 256, size=(B, K, N), dtype=np.int32).astype(np.uint8)
    out = kernel(x, y, np.float32(0.03), np.float32(0.025))
    ref = np.einsum(
        "gbmk,bkn->gbmn",
        (x.astype(np.float32) + 66.0) * 0.03,
        (y.astype(np.float32) - 160.0) * 0.025,
    )
    err = np.abs(out - ref).max() / max(np.abs(ref).max(), 1e-9)
    print("max rel err:", err)
